# revision 2
# baseline (speedup 1.0000x reference)
"""Trainium2 Bass kernel for multi-head attention (B=4, N=2048, C=1024, H=16).

Sharding: 8 cores = (batch b, query-half qh). Each core computes attention for
its 1024 query tokens of batch b against all 2048 keys of batch b, all 16
heads, plus the output projection. Host-side work is layout only (transpose /
concat); all FLOPs run on device.

Per-core layout: activations are feature-major ("xT" = [C, tokens]) so every
matmul contracts over the partition axis. Scores are computed transposed
(ST[j keys, i queries]) which makes softmax denominators a ones-matmul and
makes P@V need no transpose of P. Softmax skips max-subtraction (|S| <~ 25
for this distribution, exp is safe in fp32). All matmul operands are bitcast
to float32r (FP22) which streams at 1 cycle/row for free dims >= 256.
"""

import sys

import ml_dtypes
import numpy as np

BF = ml_dtypes.bfloat16

sys.path.insert(0, "/opt/trn_rl_repo")

import concourse.bass as bass  # noqa: E402
import concourse.bacc as bacc  # noqa: E402
import concourse.mybir as mybir  # noqa: E402
import concourse.tile as tile  # noqa: E402

# Problem constants (hardcoded by contract).
B, N, C = 4, 2048, 1024
H, D = 16, 64
SCALE = D**-0.5  # 0.125
T = N  # key tokens per core
TQ = N // 2  # query tokens per core
KC = C // 128  # contraction chunks of 128
PAIRS = H // 2  # head pairs; pair g = heads (2g, 2g+1)
NJT = T // 128  # 16 key tiles
F32 = mybir.dt.float32
F32R = mybir.dt.float32r
BF16 = mybir.dt.bfloat16
EXP = mybir.ActivationFunctionType.Exp




def build_bass(reps=1, loop_iters=0, ablate=()):
    from contextlib import ExitStack

    nc = bacc.Bacc()
    xT = nc.dram_tensor("xT", [C, T], BF16, kind="ExternalInput")
    wkT = nc.dram_tensor("wkT", [PAIRS, 128, KC, 128], BF16, kind="ExternalInput")
    wqT = nc.dram_tensor("wqT", [PAIRS, 128, KC, 128], BF16, kind="ExternalInput")
    wvT = nc.dram_tensor("wvT", [PAIRS // 2, 128, KC, 260], BF16, kind="ExternalInput")
    woT = nc.dram_tensor("woT", [128, KC, C], BF16, kind="ExternalInput")
    bo = nc.dram_tensor("bo", [C], F32, kind="ExternalInput")
    ones_in = nc.dram_tensor("ones_in", [128, 128], BF16, kind="ExternalInput")
    outT = nc.dram_tensor("outT", [C, TQ], F32, kind="ExternalOutput")

    xT_r = xT.rearrange("(kc p) t -> p kc t", p=128)
    bo_r = bo.rearrange("(a p) -> p a", p=128)
    outT_r = outT.rearrange("(et p) i -> et p i", p=128)

    with tile.TileContext(nc) as tc, ExitStack() as ctx:
        ctx.enter_context(
            nc.allow_low_precision(reason="float32r tiles: FP22 is within error budget")
        )
        const = ctx.enter_context(tc.tile_pool(name="const", bufs=1))
        wpool = ctx.enter_context(tc.tile_pool(name="wpool", bufs=2))
        kqp = ctx.enter_context(tc.tile_pool(name="kqp", bufs=2))
        vpool = ctx.enter_context(tc.tile_pool(name="vpool", bufs=2))
        ptp = ctx.enter_context(tc.tile_pool(name="ptp", bufs=2))
        smp = ctx.enter_context(tc.tile_pool(name="smp", bufs=1))
        obp = ctx.enter_context(tc.tile_pool(name="obp", bufs=2))
        ps = ctx.enter_context(tc.tile_pool(name="ps", bufs=4, space="PSUM"))
        ps2 = ctx.enter_context(tc.tile_pool(name="ps2", bufs=2, space="PSUM"))
        drp = ctx.enter_context(tc.tile_pool(name="drp", bufs=2, space="DRAM"))

        import contextlib
        loop_ctx = (
            tc.For_i(0, loop_iters, 1) if loop_iters else contextlib.nullcontext()
        )
        with loop_ctx:
          for _rep in range(reps):
            # Constants first (small), then xT per c-chunk so the first
            # projection matmuls start as soon as chunk 0 lands — a weight DMA
            # queued behind the full 8MB xT would stall the PE ~30us.
            ones = const.tile([128, 128], BF16, tag="ones")
            nc.sync.dma_start(out=ones, in_=ones_in[:, :])
            bo_t = const.tile([128, KC], F32, tag="bo")
            nc.sync.dma_start(out=bo_t, in_=bo_r)
            xt = const.tile([128, KC, T], BF16, tag="xw")
            # O^T, concatenated over heads: rows fc*128+p = feature f, cols = query i.
            ot_t = const.tile([128, PAIRS, TQ], BF16, tag="ot")

            def kq_alloc(g):
                """Allocate tiles and start weight DMAs for pair g's k/q projections."""
                t = {}
                t["wk"] = wpool.tile([128, KC, 128], BF16, tag="wk", name=f"wk{g}")
                nc.sync.dma_start(out=t["wk"], in_=wkT[g])
                t["wq"] = wpool.tile([128, KC, 128], BF16, tag="wq", name=f"wq{g}")
                nc.sync.dma_start(out=t["wq"], in_=wqT[g])
                t["kt"] = kqp.tile([128, T], BF16, tag="kT", name=f"kt{g}")
                t["qt"] = kqp.tile([128, TQ], BF16, tag="qT", name=f"qt{g}")
                return t

            def v_alloc(p):
                """V tiles for pair group p (pairs 2p, 2p+1), with ones columns."""
                t = {}
                t["wv"] = wpool.tile([128, KC, 260], BF16, tag="wv", name=f"wv{p}")
                nc.sync.dma_start(out=t["wv"], in_=wvT[p])
                t["v"] = vpool.tile([128, NJT, 260], BF16, tag="v", name=f"v{p}")
                return t

            def v_emit(t):
                if "proj" in ablate:
                    nc.vector.tensor_copy(t["v"][:, 0, 0:256], xt[:, 0, 0:256])
                    yield
                    return
                wv_t, v_t = t["wv"], t["v"]
                for tt in range(NJT):
                    pv = ps.tile([128, 512], F32, tag="ps", name="pv")
                    for kc in range(KC):
                        nc.tensor.matmul(
                            pv[:, 0:260],
                            xt[:, kc, tt * 128 : (tt + 1) * 128],
                            wv_t[:, kc, :],
                            start=(kc == 0),
                            stop=(kc == KC - 1),
                        )
                        yield
                    nc.vector.tensor_copy(v_t[:, tt, :], pv[:, 0:260])
                    yield
                # Overwrite each head's pad column with ones (softmax denom).
                v_ones = v_t.rearrange("p t (h c) -> p t h c", c=65)[:, :, :, 64]
                nc.vector.tensor_copy(
                    v_ones, ones[:, 0:64].rearrange("p (t h) -> p t h", h=4)
                )
                yield

            def kq_emit(g, t):
                if "proj" in ablate:
                    nc.vector.tensor_copy(t["kt"][:, 0:512], xt[:, 0, 0:512])
                    nc.vector.tensor_copy(t["qt"][:, 0:512], xt[:, 1, 0:512])
                    yield
                    return
                kt_t = t["kt"]
                for tcn in range(T // 512):
                    pk = ps.tile([128, 512], F32, tag="ps", name=f"pk{g}_{tcn}")
                    for kc in range(KC):
                        nc.tensor.matmul(
                            pk,
                            t["wk"][:, kc, :],
                            xt[:, kc, tcn * 512 : (tcn + 1) * 512],
                            start=(kc == 0),
                            stop=(kc == KC - 1),
                        )
                        yield
                    nc.vector.tensor_copy(kt_t[:, tcn * 512 : (tcn + 1) * 512], pk)
                    yield
                qt_t = t["qt"]
                for icn in range(TQ // 512):
                    pq = ps.tile([128, 512], F32, tag="ps", name=f"pq{g}_{icn}")
                    for kc in range(KC):
                        nc.tensor.matmul(
                            pq,
                            t["wq"][:, kc, :],
                            xt[:, kc, icn * 512 : (icn + 1) * 512],
                            start=(kc == 0),
                            stop=(kc == KC - 1),
                        )
                        yield
                    nc.vector.tensor_copy(qt_t[:, icn * 512 : (icn + 1) * 512], pq)
                    yield

            # Pair 0 runs up front; pair g+1's k/q projections and the next
            # group's V projection are interleaved into pair g's attention so the
            # PE fills otherwise ACT-bound stretches.
            tiles = [None] * PAIRS
            vt = [None] * (PAIRS // 2)
            tiles[0] = kq_alloc(0)
            vt[0] = v_alloc(0)
            for kc in range(KC):
                nc.sync.dma_start(out=xt[:, kc, :], in_=xT_r[:, kc, :])
            for _ in v_emit(vt[0]):
                pass
            for _ in kq_emit(0, tiles[0]):
                pass

            v_gen = iter(())
            for g in range(PAIRS):
                t = tiles[g]
                kt_t, qt_t = t["kt"], t["qt"]
                v_t = vt[g // 2]["v"]
                vcol = (g % 2) * 130
                if g + 1 < PAIRS:
                    tiles[g + 1] = kq_alloc(g + 1)
                    kq_gen = kq_emit(g + 1, tiles[g + 1])
                else:
                    kq_gen = iter(())
                if g % 2 == 0 and g // 2 + 1 < PAIRS // 2:
                    vt[g // 2 + 1] = v_alloc(g // 2 + 1)
                    v_gen = v_emit(vt[g // 2 + 1])

                if "att" in ablate:
                    for _ in kq_gen:
                        pass
                    for _ in v_gen:
                        pass
                    nc.vector.tensor_copy(ot_t[0:64, g, 0:512], kt_t[0:64, 0:512])
                    continue
                for icn in range(TQ // 512):
                    isl = slice(icn * 512, (icn + 1) * 512)
                    # O^T accumulators per head, [65, 512]: rows 0:64 = output,
                    # row 64 = softmax denominator (ones column of v).
                    av_a = ps.tile([65, 512], F32, tag="ps", name=f"ava{g}_{icn}")
                    av_b = ps.tile([65, 512], F32, tag="ps", name=f"avb{g}_{icn}")
                    prev = None
                    for jt in range(NJT):
                        jsl = slice(jt * 128, (jt + 1) * 128)
                        # S^T[j, i] for both heads into one 2-bank PSUM tile;
                        # heads packed into PE row groups 0:64 / 64:128.
                        st2 = ps2.tile([128, 2, 512], F32, tag="st2", name=f"st{g}_{icn}_{jt}")
                        nc.tensor.matmul(st2[:, 0, :], kt_t[0:64, jsl], qt_t[0:64, isl])
                        nc.tensor.matmul(st2[:, 1, :], kt_t[64:128, jsl], qt_t[64:128, isl])
                        # One exp instruction covers both heads (1024 free).
                        pt2 = ptp.tile([128, 2, 512], BF16, tag="pt", name=f"pt{g}_{icn}_{jt}")
                        nc.scalar.activation(pt2[:, :, :], st2[:, :, :], EXP, scale=SCALE)
                        # Interleave next projections' matmuls while ACT runs.
                        for _ in range(2):
                            next(kq_gen, None)
                        for _ in range(3):
                            next(v_gen, None)
                        # AV lags one step so its exp input is ready when PE gets here.
                        if prev is not None:
                            nc.tensor.matmul(
                                av_a, v_t[:, prev[1], vcol : vcol + 65], prev[0][:, 0, :],
                                start=(prev[1] == 0), stop=False,
                            )
                            nc.tensor.matmul(
                                av_b, v_t[:, prev[1], vcol + 65 : vcol + 130], prev[0][:, 1, :],
                                start=(prev[1] == 0), stop=False,
                            )
                        prev = (pt2, jt)
                    pt2, jt_last = prev
                    nc.tensor.matmul(
                        av_a, v_t[:, jt_last, vcol : vcol + 65], pt2[:, 0, :],
                        start=False, stop=True,
                    )
                    nc.tensor.matmul(
                        av_b, v_t[:, jt_last, vcol + 65 : vcol + 130], pt2[:, 1, :],
                        start=False, stop=True,
                    )
                    # Softmax denominators sit on partition 64 of each
                    # accumulator. Reciprocal there, bounce through DRAM, and
                    # broadcast-load across 64 partitions (stride-0 DRAM AP) —
                    # the epilogue has no PE instructions at all.
                    rec_a = smp.tile([65, 512], F32, tag="rA", name=f"ra{g}_{icn}")
                    nc.vector.reciprocal(rec_a[64:65, :], av_a[64:65, :])
                    rec_b = smp.tile([65, 512], F32, tag="rB", name=f"rb{g}_{icn}")
                    nc.vector.reciprocal(rec_b[64:65, :], av_b[64:65, :])
                    rec_d = drp.tile([2, 512], F32, tag="rd", name=f"rd{g}_{icn}")
                    nc.sync.dma_start(out=rec_d[0:1, :], in_=rec_a[64:65, :])
                    nc.sync.dma_start(out=rec_d[1:2, :], in_=rec_b[64:65, :])
                    bc_sba = smp.tile([64, 512], F32, tag="bcA", name=f"bsa{g}_{icn}")
                    nc.sync.dma_start(
                        out=bc_sba,
                        in_=bass.AP(tensor=rec_d[:, :].tensor, offset=rec_d[0:1, :].offset,
                                    ap=[[0, 64], [1, 512]]),
                    )
                    bc_sbb = smp.tile([64, 512], F32, tag="bcB", name=f"bsb{g}_{icn}")
                    nc.sync.dma_start(
                        out=bc_sbb,
                        in_=bass.AP(tensor=rec_d[:, :].tensor, offset=rec_d[1:2, :].offset,
                                    ap=[[0, 64], [1, 512]]),
                    )
                    # Normalize. Head A lands in ot rows 0:64 directly; head B
                    # is staged and DMA'd across partitions into rows 64:128.
                    nc.vector.tensor_mul(ot_t[0:64, g, isl], av_a[0:64, :], bc_sba)
                    ot_bst = smp.tile([64, 512], BF16, tag="otB", name=f"ob{g}_{icn}")
                    nc.vector.tensor_mul(ot_bst, av_b[0:64, :], bc_sbb)
                    nc.sync.dma_start(out=ot_t[64:128, g, isl], in_=ot_bst)
                for _ in kq_gen:
                    pass
                if g % 2 == 1:
                    for _ in v_gen:
                        pass

            if "out" in ablate:
                ob0 = obp.tile([128, 512], F32, tag="ob", name="ob0")
                nc.vector.tensor_copy(ob0, ot_t[:, 0, 0:512])
                nc.sync.dma_start(out=outT_r[0, :, 0:512], in_=ob0)
                continue
            # Output projection: outT[e, i] = Wo @ O^T + bo.
            wo_t = const.tile([128, KC, C], BF16, tag="xw")
            nc.sync.dma_start(out=wo_t, in_=woT[:, :, :])
            for et in range(C // 128):
                for icn in range(TQ // 512):
                    po = ps.tile([128, 512], F32, tag="ps", name=f"po{et}_{icn}")
                    for fc in range(KC):
                        nc.tensor.matmul(
                            po,
                            wo_t[:, fc, et * 128 : (et + 1) * 128],
                            ot_t[:, fc, icn * 512 : (icn + 1) * 512],
                            start=(fc == 0),
                            stop=(fc == KC - 1),
                        )
                    ob = obp.tile([128, 512], F32, tag="ob", name=f"o{et}_{icn}")
                    nc.vector.tensor_scalar_add(ob, po, bo_t[:, et : et + 1])
                    nc.sync.dma_start(
                        out=outT_r[et, :, icn * 512 : (icn + 1) * 512], in_=ob
                    )

    nc.finalize()
    return nc


_CACHE = {}


def _get_nc():
    if "nc" not in _CACHE:
        _CACHE["nc"] = build_bass()
    return _CACHE["nc"]


def make_in_maps(x, Wq, Wk, Wv, Wo, bo):
    """Host-side sharding: layout prep only (transposes / concatenation)."""
    x = np.asarray(x, dtype=np.float32)
    # Weights pre-tiled into the exact SBUF layouts (contiguous DMAs).
    # wk/wq: [g, p, kc, o] = W[g*128+o, kc*128+p]
    wkT = np.ascontiguousarray(
        np.asarray(Wk, np.float32).reshape(PAIRS, 128, KC, 128).transpose(0, 3, 2, 1)
    ).astype(BF)
    wqT = np.ascontiguousarray(
        np.asarray(Wq, np.float32).reshape(PAIRS, 128, KC, 128).transpose(0, 3, 2, 1)
    ).astype(BF)
    # wv: zero-pad a column after each head's 64, then [grp, p, kc, col]
    wv_pad = np.zeros((C, 1040), np.float32)
    wvT_raw = np.asarray(Wv, np.float32).T  # [c, o]
    for h in range(H):
        wv_pad[:, h * 65 : h * 65 + 64] = wvT_raw[:, h * 64 : (h + 1) * 64]
    wvT = np.ascontiguousarray(
        wv_pad.reshape(KC, 128, 4, 260).transpose(2, 1, 0, 3)
    ).astype(BF)
    # wo: [p, fc, e] = Wo[e, fc*128+p]
    woT = np.ascontiguousarray(
        np.asarray(Wo, np.float32).T.reshape(KC, 128, C).transpose(1, 0, 2)
    ).astype(BF)
    bo = np.ascontiguousarray(np.asarray(bo, np.float32))
    in_maps = []
    for core in range(8):
        b, qh = core // 2, core % 2
        xb = x[b]
        # My query half first; key/value order is permutation-invariant.
        xrot = np.concatenate([xb[qh * TQ : (qh + 1) * TQ], xb[(1 - qh) * TQ : (2 - qh) * TQ]], axis=0)
        xT_np = np.ascontiguousarray(xrot.T).astype(BF)
        in_maps.append(
            {
                "xT": xT_np,
                "wkT": wkT,
                "wqT": wqT,
                "wvT": wvT,
                "woT": woT,
                "bo": bo,
                "ones_in": np.ones((128, 128), BF),
            }
        )
    return in_maps


def gather_out(results):
    out = np.empty((B, N, C), dtype=np.float32)
    for core in range(8):
        b, qh = core // 2, core % 2
        out[b, qh * TQ : (qh + 1) * TQ, :] = results[core]["outT"].T
    return out


def kernel(x, Wq, Wk, Wv, Wo, bo):
    from concourse.bass_utils import run_bass_kernel_spmd

    in_maps = make_in_maps(x, Wq, Wk, Wv, Wo, bo)
    res = run_bass_kernel_spmd(_get_nc(), in_maps, core_ids=list(range(8)))
    return gather_out(res.results)

